# revision 6
# baseline (speedup 1.0000x reference)
"""Trainium2 Bass kernel for nn_Attention_cross (dual-branch cross-reuse attention).

Reference computation (B=4, N0=2048, C=768, H=12, hd=64, N=1024):
  x_diff, x_cond = x[:, :N], x[:, N:]
  q,k,v per branch = x @ w_qkv (per-head), attn = softmax(q k^T / sqrt(hd))
  o_d = ((attn_diff @ v_d) @ w_proj_diff + b_d) reused per-head with attn_cond
  o_c = (attn_cond @ v_c) @ w_proj_cond + b_c
  out = concat([o_d, o_c], axis=1)

Sharding: 8 cores = 4 batches x 2 head-groups (6 heads each). The
head-mixing projections are row-sharded with a pair AllReduce.
All matmuls in float32r (full PE rate). Attention computed fully
transposed (S^T = K Q^T); softmax sums come from a ones-column in V.
"""
import numpy as np

import concourse.bass as bass
import concourse.tile as tile
from concourse import bacc, mybir
from concourse.bass_utils import run_bass_kernel_spmd
from concourse.masks import make_identity

F32 = mybir.dt.float32
F32R = mybir.dt.float32r
Exp = mybir.ActivationFunctionType.Exp

B, N0, C = 4, 2048, 768
H, HD = 12, 64
N = N0 // 2              # 1024 sequence per branch
HPC = H // 2             # 6 heads per core
CW = HPC * HD            # 384 own C-columns/rows
NCH = N // 128           # 8 chunks of 128 along n/m
CCH = C // 128           # 6 chunks of 128 along C
NB = N // 512            # 2 blocks of 512 along n
SCALE = HD ** -0.5

N_CORES = 8
GROUPS = [[0, 1], [2, 3], [4, 5], [6, 7]]

_CACHE = {}


def _build():
    nc = bacc.Bacc("TRN2", target_bir_lowering=False, debug=False,
                   num_devices=N_CORES)

    x_b = nc.dram_tensor("x_b", [N0, C], F32, kind="ExternalInput").ap()
    wqk_d = nc.dram_tensor("wqk_d", [C, 2 * CW], F32, kind="ExternalInput").ap()
    wqk_c = nc.dram_tensor("wqk_c", [C, 2 * CW], F32, kind="ExternalInput").ap()
    wv_d = nc.dram_tensor("wv_d", [C, CW], F32, kind="ExternalInput").ap()
    wv_c = nc.dram_tensor("wv_c", [C, CW], F32, kind="ExternalInput").ap()
    wp_d = nc.dram_tensor("wp_d", [CW, C], F32, kind="ExternalInput").ap()
    wp_c = nc.dram_tensor("wp_c", [CW, C], F32, kind="ExternalInput").ap()
    bias_d = nc.dram_tensor("bias_d", [1, CW], F32, kind="ExternalInput").ap()
    bias_c = nc.dram_tensor("bias_c", [1, CW], F32, kind="ExternalInput").ap()
    o_d_cols = nc.dram_tensor("o_d_cols", [N, CW], F32, kind="ExternalOutput").ap()
    o_c_full = nc.dram_tensor("o_c_cols", [N, CW], F32, kind="ExternalOutput").ap()

    # ReduceScatter: input [2, N, CW] slot-major (slot = head-group), each core
    # of the pair receives its own slot reduced: out [N, CW]
    cc_in_d = nc.dram_tensor("cc_in_d", [2, N, CW], F32).ap()
    cc_out_d = nc.dram_tensor("cc_out_d", [N, CW], F32).ap()
    cc_in_c = nc.dram_tensor("cc_in_c", [2, N, CW], F32).ap()
    cc_out_c = nc.dram_tensor("cc_out_c", [N, CW], F32).ap()

    with tile.TileContext(nc) as tc:
        _body(nc, tc, x_b, wqk_d, wqk_c, wv_d, wv_c, wp_d, wp_c,
              bias_d, bias_c, o_d_cols, o_c_full,
              cc_in_d, cc_out_d, cc_in_c, cc_out_c)
    nc.compile()
    return nc


def _body(nc, tc, x_b, wqk_d, wqk_c, wv_d, wv_c, wp_d, wp_c,
          bias_d, bias_c, o_d_cols, o_c_full,
          cc_in_d, cc_out_d, cc_in_c, cc_out_c):
    from contextlib import ExitStack
    ctx = ExitStack()
    with ctx:
        ctx.enter_context(nc.allow_low_precision(reason="fp32r matmul inputs by design"))
        singles = ctx.enter_context(tc.tile_pool(name="singles", bufs=1))
        ident = singles.tile([128, 128], F32)
        make_identity(nc, ident[:])
        ones_r = singles.tile([1, 128], F32R)
        nc.vector.memset(ones_r[:].bitcast(F32), 1.0)

        big = ctx.enter_context(tc.tile_pool(name="big", bufs=1))
        qkT = {}    # branch -> [128, 6, N] fp32r  rows: [q h0..h5 | k h0..h5]
        v_aug = {}  # branch -> [128, HPC, NCH, 65] fp32r
        u_t = {}    # branch -> [128, 3, N] fp32r (normalized (attn@v)^T)
        for br in ("d", "c"):
            qkT[br] = big.tile([128, 2 * CW // 128, N], F32R, tag=f"qkT_{br}", name=f"qkT_{br}")
            v_aug[br] = big.tile([128, HPC, NCH, HD + 1], F32R, tag=f"v_{br}", name=f"v_{br}")
            nc.vector.memset(v_aug[br][:, :, :, HD:HD + 1].bitcast(F32), 1.0)
        p_d = big.tile([128, NCH, CW], F32R, tag="p_d")  # own proj_diff cols post-reducescatter

        # ---------- Stage A+B: transpose x, QKV ----------
        with tc.tile_pool(name="ab", bufs=2) as ab, \
             tc.tile_pool(name="wst", bufs=1) as wst, \
             tc.tile_pool(name="xt_pool", bufs=1) as xt_pool, \
             tc.tile_pool(name="ps_ab", bufs=2, space="PSUM") as ps_ab:
            for bi, (br, wqk, wv) in enumerate(
                    [("d", wqk_d, wv_d), ("c", wqk_c, wv_c)]):
                half = bi * N
                xT = xt_pool.tile([128, CCH, N], F32R, tag="xT")
                for j in range(NCH):
                    xn = ab.tile([128, C], F32, tag="x_nat")
                    nc.sync.dma_start(
                        out=xn[:], in_=x_b[half + j * 128: half + (j + 1) * 128, :])
                    for i in range(CCH):
                        tp = ps_ab.tile([128, 128], F32, tag="tp")
                        nc.tensor.transpose(tp[:], xn[:, i * 128:(i + 1) * 128], ident[:])
                        nc.vector.tensor_copy(xT[:, i, j * 128:(j + 1) * 128], tp[:])

                wqk_r = xt_pool.tile([128, CCH, 2 * CW], F32R, tag="wqk_r")
                wv_r = xt_pool.tile([128, CCH, CW], F32R, tag="wv_r")
                for i in range(CCH):
                    ws = wst.tile([128, 2 * CW], F32, tag="w_stage")
                    nc.sync.dma_start(out=ws[:], in_=wqk[i * 128:(i + 1) * 128, :])
                    nc.vector.tensor_copy(wqk_r[:, i, :], ws[:])
                    ws2 = wst.tile([128, CW], F32, tag="w_stage2")
                    nc.sync.dma_start(out=ws2[:], in_=wv[i * 128:(i + 1) * 128, :])
                    nc.vector.tensor_copy(wv_r[:, i, :], ws2[:])

                for fi in range(2 * CW // 128):
                    for nb in range(NB):
                        ps = ps_ab.tile([128, 512], F32, tag="qk_ps")
                        for ci in range(CCH):
                            nc.tensor.matmul(
                                ps[:],
                                wqk_r[:, ci, fi * 128:(fi + 1) * 128],
                                xT[:, ci, nb * 512:(nb + 1) * 512],
                                start=(ci == 0), stop=(ci == CCH - 1))
                        nc.vector.tensor_copy(
                            qkT[br][:, fi, nb * 512:(nb + 1) * 512], ps[:])

                for mch in range(NCH):
                    ps = ps_ab.tile([128, CW], F32, tag="v_ps")
                    for ci in range(CCH):
                        nc.tensor.matmul(
                            ps[:], xT[:, ci, mch * 128:(mch + 1) * 128],
                            wv_r[:, ci, :],
                            start=(ci == 0), stop=(ci == CCH - 1))
                    for h in range(HPC):
                        nc.vector.tensor_copy(
                            v_aug[br][:, h, mch, 0:HD],
                            ps[:, h * HD:(h + 1) * HD])

        # ---------- attention head helper (nb-granular) ----------
        def attn_head(br, h, eT_pool, small, ps_sc, ps_av,
                      extra_lhs=None, extra_out_cb=None):
            qc, qo = divmod(h * HD, 128)
            kc, ko = divmod(CW + h * HD, 128)
            uc, uo = divmod(h * HD, 128)
            for nb in range(NB):
                eT = eT_pool.tile([128, NCH, 512], F32R, tag="eT")
                for mch in range(NCH):
                    ps = ps_sc.tile([128, 512], F32, tag="sc_ps")
                    nc.tensor.matmul(
                        ps[:],
                        qkT[br][ko:ko + HD, kc, mch * 128:(mch + 1) * 128],
                        qkT[br][qo:qo + HD, qc, nb * 512:(nb + 1) * 512],
                        start=True, stop=True)
                    nc.scalar.activation(eT[:, mch, :], ps[:], Exp)
                # u_un^T[65, 512] = v_aug.T @ eT (row 64 = sums)
                ps_u = ps_av.tile([HD + 1, 512], F32, tag="av_ps")
                for mch in range(NCH):
                    nc.tensor.matmul(
                        ps_u[:], v_aug[br][:, h, mch, :], eT[:, mch, :],
                        start=(mch == 0), stop=(mch == NCH - 1))
                r_t = small.tile([1, 512], F32R, tag="r_t")
                nc.vector.reciprocal(r_t[:], ps_u[HD:HD + 1, :])
                ps_b = ps_av.tile([HD, 512], F32, tag="rb_ps")
                nc.tensor.matmul(ps_b[:], ones_r[:, 0:HD], r_t[:],
                                 start=True, stop=True)
                r_bs = small.tile([HD, 512], F32, tag="r_bs")
                nc.vector.tensor_copy(r_bs[:], ps_b[:])
                nc.vector.tensor_mul(
                    u_t[br][uo:uo + HD, uc, nb * 512:(nb + 1) * 512],
                    ps_u[0:HD, :], r_bs[:])
                if extra_lhs is not None:
                    ps_o = ps_av.tile([HD, 512], F32, tag="o2_ps")
                    for mch in range(NCH):
                        nc.tensor.matmul(
                            ps_o[:], extra_lhs(mch, h), eT[:, mch, :],
                            start=(mch == 0), stop=(mch == NCH - 1))
                    extra_out_cb(nb, ps_o, r_bs)

        def proj_partial(u, wp_dram, cc_in, tagp):
            with tc.tile_pool(name=f"proj_{tagp}", bufs=2) as pj, \
                 tc.tile_pool(name=f"wpp_{tagp}", bufs=1) as wpp, \
                 tc.tile_pool(name=f"ps_pj_{tagp}", bufs=2, space="PSUM") as ps_pj:
                wp_r = wpp.tile([128, CW // 128, C], F32R, tag="wp_r")
                for i in range(CW // 128):
                    ws = pj.tile([128, C], F32, tag="wp_st")
                    nc.sync.dma_start(out=ws[:], in_=wp_dram[i * 128:(i + 1) * 128, :])
                    nc.vector.tensor_copy(wp_r[:, i, :], ws[:])
                for nch in range(NCH):
                    for slot in range(2):
                        ps = ps_pj.tile([128, CW], F32, tag="pj_ps")
                        for ci in range(CW // 128):
                            nc.tensor.matmul(
                                ps[:],
                                u[:, ci, nch * 128:(nch + 1) * 128],
                                wp_r[:, ci, slot * CW:(slot + 1) * CW],
                                start=(ci == 0), stop=(ci == CW // 128 - 1))
                        st = pj.tile([128, CW], F32, tag="pj_st")
                        nc.vector.tensor_copy(st[:], ps[:])
                        nc.sync.dma_start(
                            out=cc_in[slot, nch * 128:(nch + 1) * 128, :],
                            in_=st[:])

        with tc.tile_pool(name="eT_pool", bufs=2) as eT_pool, \
             tc.tile_pool(name="small", bufs=4) as small, \
             tc.tile_pool(name="upool", bufs=1) as upool, \
             tc.tile_pool(name="ps_sc", bufs=2, space="PSUM") as ps_sc, \
             tc.tile_pool(name="ps_av", bufs=1, space="PSUM") as ps_av:

            # ---------- Stage C: diff attention ----------
            u_t["d"] = upool.tile([128, CW // 128, N], F32R, tag="u", name="u_d")
            for h in range(HPC):
                attn_head("d", h, eT_pool, small, ps_sc, ps_av)

            # ---------- Stage D: proj_diff partial + AllReduce ----------
            proj_partial(u_t["d"], wp_d, cc_in_d, "d")
            nc.gpsimd.collective_compute(
                "ReduceScatter", mybir.AluOpType.add, replica_groups=GROUPS,
                ins=[cc_in_d], outs=[cc_out_d])
            with tc.tile_pool(name="pd_load", bufs=2) as pdl:
                bias_bd = pdl.tile([128, CW], F32, tag="bias_b")
                nc.sync.dma_start(out=bias_bd[:], in_=bias_d.to_broadcast([128, CW]))
                for mch in range(NCH):
                    pin = pdl.tile([128, CW], F32, tag="p_in")
                    nc.sync.dma_start(
                        out=pin[:], in_=cc_out_d[mch * 128:(mch + 1) * 128, :])
                    nc.vector.tensor_add(p_d[:, mch, :], pin[:], bias_bd[:])

            # ---------- Stage E: cond attention + second attention ----------
            def p_d_lhs(mch, h):
                return p_d[:, mch, h * HD:(h + 1) * HD]

            u_t["c"] = upool.tile([128, CW // 128, N], F32R, tag="u", name="u_c")
            with tc.tile_pool(name="o2pool", bufs=2) as o2p, \
                 tc.tile_pool(name="ps_o2", bufs=2, space="PSUM") as ps_o2:
                for h in range(HPC):
                    o2T = o2p.tile([HD, N], F32, tag="o2T")

                    def o2_cb(nb, ps_o, r_bs, o2T=o2T):
                        nc.vector.tensor_mul(
                            o2T[:, nb * 512:(nb + 1) * 512], ps_o[:], r_bs[:])

                    attn_head("c", h, eT_pool, small, ps_sc, ps_av,
                              extra_lhs=p_d_lhs, extra_out_cb=o2_cb)
                    for nch in range(NCH):
                        tp = ps_o2.tile([128, HD], F32, tag="o2_tp")
                        nc.tensor.matmul(
                            tp[:], o2T[:, nch * 128:(nch + 1) * 128],
                            ident[0:HD, 0:HD], is_transpose=True)
                        on = o2p.tile([128, HD], F32, tag="o2_nat")
                        nc.vector.tensor_copy(on[:], tp[:])
                        nc.sync.dma_start(
                            out=o_d_cols[nch * 128:(nch + 1) * 128,
                                         h * HD:(h + 1) * HD],
                            in_=on[:])

            # ---------- Stage F: proj_cond + AllReduce + o_c ----------
            proj_partial(u_t["c"], wp_c, cc_in_c, "c")
            nc.gpsimd.collective_compute(
                "ReduceScatter", mybir.AluOpType.add, replica_groups=GROUPS,
                ins=[cc_in_c], outs=[cc_out_c])
            with tc.tile_pool(name="oc_store", bufs=2) as ocs:
                bias_bc = ocs.tile([128, CW], F32, tag="bias_bc")
                nc.sync.dma_start(out=bias_bc[:], in_=bias_c.to_broadcast([128, CW]))
                for mch in range(NCH):
                    pin = ocs.tile([128, CW], F32, tag="pc_in")
                    nc.sync.dma_start(
                        out=pin[:], in_=cc_out_c[mch * 128:(mch + 1) * 128, :])
                    ob = ocs.tile([128, CW], F32, tag="oc_out")
                    nc.vector.tensor_add(ob[:], pin[:], bias_bc[:])
                    nc.sync.dma_start(
                        out=o_c_full[mch * 128:(mch + 1) * 128, :], in_=ob[:])


def _prep_inputs(x, w_qkv_diff, w_qkv_cond, w_proj_diff, b_proj_diff,
                 w_proj_cond, b_proj_cond):
    in_maps = []
    for c in range(N_CORES):
        b, hg = divmod(c, 2)
        s = slice(hg * CW, (hg + 1) * CW)
        sk = slice(C + hg * CW, C + (hg + 1) * CW)
        sv = slice(2 * C + hg * CW, 2 * C + (hg + 1) * CW)
        m = {
            "x_b": x[b],
            "wqk_d": np.concatenate([w_qkv_diff[:, s] * SCALE, w_qkv_diff[:, sk]], axis=1),
            "wqk_c": np.concatenate([w_qkv_cond[:, s] * SCALE, w_qkv_cond[:, sk]], axis=1),
            "wv_d": w_qkv_diff[:, sv],
            "wv_c": w_qkv_cond[:, sv],
            "wp_d": w_proj_diff[s, :],
            "wp_c": w_proj_cond[s, :],
            "bias_d": b_proj_diff[None, s],
            "bias_c": b_proj_cond[None, s],
        }
        in_maps.append({k: np.ascontiguousarray(v, np.float32) for k, v in m.items()})
    return in_maps


def kernel(x, w_qkv_diff, w_qkv_cond, w_proj_diff, b_proj_diff,
           w_proj_cond, b_proj_cond):
    x = np.asarray(x)
    w_qkv_diff = np.asarray(w_qkv_diff)
    w_qkv_cond = np.asarray(w_qkv_cond)
    w_proj_diff = np.asarray(w_proj_diff)
    b_proj_diff = np.asarray(b_proj_diff)
    w_proj_cond = np.asarray(w_proj_cond)
    b_proj_cond = np.asarray(b_proj_cond)

    if "nc" not in _CACHE:
        _CACHE["nc"] = _build()
    nc = _CACHE["nc"]
    in_maps = _prep_inputs(x, w_qkv_diff, w_qkv_cond, w_proj_diff,
                           b_proj_diff, w_proj_cond, b_proj_cond)
    res = run_bass_kernel_spmd(nc, in_maps, list(range(N_CORES))).results

    o_d = np.empty((B, N, C), np.float32)
    o_c = np.empty((B, N, C), np.float32)
    for c in range(N_CORES):
        b, hg = divmod(c, 2)
        o_d[b][:, hg * CW:(hg + 1) * CW] = res[c]["o_d_cols"]
        o_c[b][:, hg * CW:(hg + 1) * CW] = res[c]["o_c_cols"]
    return np.concatenate([o_d, o_c], axis=1)
